# revision 10
# baseline (speedup 1.0000x reference)
"""BoundaryLoss Trainium2 kernel (8 NeuronCores, SPMD, strip-replicated).

Layout: core c owns output column block [128c, 128c+128). The host hands
each core a strip of every input row covering its block plus a margin of
w columns on each side (w = bucketed max in-row nearest-background
distance, measured exactly on the host as in the previous revision).
Row-local EDT distances never exceed w at the central columns, so each
core can run the full row pass locally — no AllToAll at all, which in the
prior revision serialized ~70us of collective latency ahead of the column
pass.

Pipeline (per core):
  1. Row pass on [128, 8*W] fp16 strips (W = 128+2w; partition p, block b
     holds image row 128b+p). One forward + one reverse
     tensor_tensor_scan per image; the scan chains across block
     boundaries, but any carried-in state reaches a central column with
     value > w and so never wins (margin absorbs it).
  2. PE-transpose the central 128 columns of each block (g, fp16), square
     on the PSUM->SBUF evacuation (ACT), assembling g2^T [128 cols, 1024
     rows] directly — all overlapped with the other image's row pass.
  3. Column min-plus D2[j,i] = min_dd (dd^2 + g2T[j, i+dd]) over
     |dd| <= w on DVE in fp16 when w <= 44 (integers <= 2048 are fp16-
     exact; candidates in (2048, 4096] round by <= 1, a <= 0.05% error),
     f32 (STT pairs) otherwise. Odd shifts read a one-element-shifted
     copy to keep 4-byte alignment for the DVE 2x mode.
  4. Per-image global max via one small AllReduce (a dummy AllReduce at
     t=0 absorbs this runtime's ~55us first-collective barrier under the
     compute), then a short fp16 tail: masks compare unnormalized
     d = sqrt(D2) against 0.1*(max+1e-6), diff/abs/masked partial sums
     with fused accumulate; host sums the 8 partial pairs.
"""
import os
import sys

import numpy as np

for _p in ("/opt/trn_rl_repo", "/root/.axon_site/_ro/trn_rl_repo"):
    if os.path.isdir(_p) and _p not in sys.path:
        sys.path.append(_p)

import concourse.bacc as bacc
import concourse.tile as tile
from concourse import mybir
from concourse.bass_utils import run_bass_kernel_spmd

F32 = mybir.dt.float32
FP16 = mybir.dt.float16
I32 = mybir.dt.int32
AF = mybir.ActivationFunctionType
ALU = mybir.AluOpType
AX = mybir.AxisListType

H = 1024          # image height/width
P = 128           # partitions / rows per block / cols per core block
NB = 8            # row blocks per strip (H / P)
NCORES = 8
BIG = 1.0e4
INF = 1.0e9       # f32 sentinel
HINF = 60000.0    # fp16 sentinel (fp16 max normal is 65504)
FP16_WMAX = 44    # fp16 col pass iff w <= 44 (g^2, dd^2 <= 1936 exact)

_BUCKETS = (8, 10, 12, 14, 16, 18, 20, 22, 24, 26, 28, 32, 36, 40, 44,
            48, 56, 64, 80, 96, 128, 160, 192, 256, 320)


def _col_pass(tc, m, w, gTp, gB, persist, work):
    """Windowed min-plus; returns acc tile [P, H] (fp16 or f32).

    acc[j, i] = min_{|dd| <= w} (dd^2 + gTp[j, w + i + dd]); gTp is
    INF-padded by w on both sides. Entirely on DVE (tensor ops are
    rejected on Pool in this compiler build).
    """
    nc = tc.nc
    use16 = gB is not None

    if use16:
        def shifted(off):  # AP of width H at element offset `off` of gTp
            if off % 2 == 0:
                return gTp[:, off:off + H]
            return gB[:, off - 1:off - 1 + H]
    else:
        def shifted(off):
            return gTp[:, off:off + H]

    acc = persist.tile([P, H], FP16 if use16 else F32, tag=f"acc{m}")
    # Plain TT gets the DVE 2x mode for 16-bit and single-src TS gets 4x,
    # while the fused STT has no fast uop — so for fp16 a 3-op pairwise
    # form beats 2 STTs per dd. dd=1 folds the d=0 term.
    if use16:
        for dd in range(1, w + 1):
            tmp = work.tile([P, H], FP16, tag=f"pm{m}_{dd % 3}")
            nc.vector.tensor_tensor(tmp[:], shifted(w + dd), shifted(w - dd),
                                    ALU.min)
            nc.vector.tensor_scalar_add(tmp[:], tmp[:], float(dd * dd))
            nc.vector.tensor_tensor(
                acc[:], shifted(w) if dd == 1 else acc[:], tmp[:], ALU.min)
    else:
        for dd in range(1, w + 1):
            c = float(dd * dd)
            nc.vector.scalar_tensor_tensor(
                acc[:], shifted(w + dd), c,
                shifted(w) if dd == 1 else acc[:], ALU.add, ALU.min)
            nc.vector.scalar_tensor_tensor(
                acc[:], shifted(w - dd), c, acc[:], ALU.add, ALU.min)
    return acc


def _body(tc, w_gt, w_pred, gts, prs, partials):
    nc = tc.nc
    rg = [list(range(NCORES))]
    ws = (w_gt, w_pred)
    srcs = (gts, prs)
    use16s = tuple(w <= FP16_WMAX for w in ws)

    with tc.tile_pool(name="const", bufs=1) as const, \
         tc.tile_pool(name="work", bufs=2) as work, \
         tc.tile_pool(name="persist", bufs=1) as persist, \
         tc.tile_pool(name="ps", bufs=1, space="PSUM") as ps, \
         tc.tile_pool(name="dram", bufs=1, space="DRAM") as dram:

        # ---- input DMA (both strips in flight immediately) ----
        # The DMA trigger instructions cost ~0.7us each on the issuing
        # queue, so the two strips go out on the two HWDGE queues (SP and
        # ACT) in parallel, ahead of everything else.
        strips = []
        for m, eng in ((0, nc.sync), (1, nc.scalar)):
            wd = NB * (P + 2 * ws[m])
            s = persist.tile([P, wd], FP16, tag=f"strip{m}")
            for q in range(4):
                eng.dma_start(s[q * 32:(q + 1) * 32, :],
                              srcs[m][q * 32:(q + 1) * 32, :])
            strips.append(s)

        # ---- warm-up collective ----
        # The CC stream opens with a fixed ~21us-deep, ~35-43us barrier,
        # and the first collective starts ~11us after that barrier ends.
        # Fire a dummy AllReduce at t=0 so this floor overlaps the local
        # compute; the real AllReduce then queues right behind it. Its
        # (zero) output is max-folded into the partials AFTER the real
        # AllReduce (keeping it live without gating the real one).
        warm_in = dram.tile([1, 8], F32)
        warm_out = nc.dram_tensor("warm_out_sh", [1, 8], F32,
                                  addr_space="Shared")
        wz = work.tile([1, 8], F32, tag="wz")
        nc.vector.memset(wz[:], 0.0)
        nc.sync.dma_start(warm_in[:, :], wz[:])
        nc.gpsimd.collective_compute(
            "AllReduce", ALU.max, replica_groups=rg,
            ins=[warm_in[:, :].opt()], outs=[warm_out[:, :].opt()])

        ar_in = dram.tile([1, 8], F32)
        ar_out = nc.dram_tensor("ar_out_sh", [1, 8], F32, addr_space="Shared")
        # zero-fill the spare AllReduce lanes without touching warm_out
        nc.sync.dma_start(ar_in[0:1, 2:8], wz[0:1, 0:6])

        # ---- constants (DVE is idle while the strips stream in) ----
        io = const.tile([P, P], I32)
        nc.gpsimd.iota(io[:], [[1, P]], base=0, channel_multiplier=-1)
        ident = const.tile([P, P], F32)
        nc.vector.tensor_scalar(ident[:], io[:], 0, None, ALU.is_equal)
        identh = const.tile([P, P], FP16)
        nc.scalar.copy(identh[:], ident[:])
        ones1 = const.tile([1, P], F32)
        nc.vector.memset(ones1[:], 1.0)
        onesc = const.tile([P, 1], F32)
        nc.vector.memset(onesc[:], 1.0)
        maxwd = max(NB * (P + 2 * w) for w in ws)
        onesh = const.tile([P, maxwd], FP16)
        nc.vector.memset(onesh[:], 1.0)

        # ================= phase 1: row pass =================
        gs = []
        for m in range(2):
            w = ws[m]
            wd = NB * (P + 2 * w)
            s = strips[m]
            # foreground -> HINF, background -> 0. Host pre-scales inputs
            # by 1e30 (saturating fp16) so `> 0` is the fg test for both
            # images and fp16 underflow cannot flip tiny positives.
            z = work.tile([P, wd], FP16, tag=f"z{m}")
            nc.vector.tensor_scalar(z[:], s[:], 0.0, HINF, ALU.is_gt,
                                    ALU.mult)
            dl = work.tile([P, wd], FP16, tag=f"dl{m}")
            nc.vector.tensor_tensor_scan(dl[:], onesh[:, :wd], z[:], INF,
                                         ALU.add, ALU.min)
            dr = work.tile([P, wd], FP16, tag=f"dr{m}")
            nc.vector.tensor_tensor_scan(dr[:, ::-1], onesh[:, :wd],
                                         z[:, ::-1], INF, ALU.add, ALU.min)
            g = work.tile([P, wd], FP16, tag=f"g{m}")
            nc.vector.tensor_tensor(g[:], dl[:], dr[:], ALU.min)
            gs.append(g)

        # ============ phase 2: transpose + square into g2^T ============
        gTps = []
        gBs = []
        for m in range(2):
            w = ws[m]
            use16 = use16s[m]
            dt = FP16 if use16 else F32
            inf = HINF if use16 else INF
            gw = H + 2 * w
            gTp = persist.tile([P, gw], dt, tag=f"gtp{m}")
            nc.vector.memset(gTp[:, :w], inf)
            nc.vector.memset(gTp[:, w + H:], inf)
            W = P + 2 * w
            for b in range(NB):
                pt = ps.tile([P, P], FP16, tag="pt", bufs=4)
                nc.tensor.transpose(pt[:], gs[m][:, b * W + w:b * W + w + P],
                                    identh[:])
                nc.scalar.activation(gTp[:, w + b * P:w + (b + 1) * P], pt[:],
                                     AF.Square)
            if use16:
                # odd shifts read a one-element-shifted copy so the AP
                # stays 4-byte-aligned for the DVE 2x fp16 mode
                gB = persist.tile([P, gw], FP16, tag=f"gb{m}")
                nc.scalar.copy(gB[:, :gw - 1], gTp[:, 1:])
                nc.vector.memset(gB[:, gw - 1:], inf)
            else:
                gB = None
            gTps.append(gTp)
            gBs.append(gB)

        # ============ phase 3: column min-plus + per-image max ============
        mx12 = work.tile([P, 2], F32, tag="mx12")
        ys = []
        for m in range(2):
            acc = _col_pass(tc, m, ws[m], gTps[m], gBs[m], persist, work)
            nc.vector.reduce_max(mx12[:, m:m + 1], acc[:], axis=AX.X)
            # unnormalized distances, precomputed before the AllReduce
            y = persist.tile([P, H], FP16, tag=f"y{m}")
            nc.scalar.activation(y[:], acc[:], AF.Sqrt)
            ys.append(y)

        # ================= phase 4: global max =================
        # partition-dim max via PE transpose [128,2] -> [2,128] + free-dim
        # reduce; the warm-up AllReduce's (zero) output is DMA'd into the
        # spare lanes of ar_in to keep it live.
        pmx = ps.tile([2, P], F32, tag="pmx")
        nc.tensor.transpose(pmx[:], mx12[:], ident[:])
        mxr = work.tile([2, 1], F32, tag="mxr")
        nc.vector.reduce_max(mxr[:], pmx[:], axis=AX.X)
        nc.sync.dma_start(ar_in[0:1, 0:2], mxr[:])
        nc.gpsimd.collective_compute(
            "AllReduce", ALU.max, replica_groups=rg,
            ins=[ar_in[:, :].opt()], outs=[ar_out[:, :].opt()])
        gmx = work.tile([1, 2], F32, tag="gmx")
        nc.sync.dma_start(gmx[:], ar_out[0:1, 0:2])

        # s2 = [inv0, inv1] with inv = 1/(sqrt(max)+1e-6); masks use
        # min(a0, a1) < 0.1 which equals (a0 < 0.1) | (a1 < 0.1).
        msq = work.tile([1, 2], F32, tag="msq")
        nc.scalar.activation(msq[:], gmx[:], AF.Sqrt)
        nc.vector.tensor_scalar_add(msq[:], msq[:], 1e-6)
        s2 = work.tile([1, 2], F32, tag="s2")
        nc.vector.reciprocal(s2[:], msq[:])
        # broadcast across partitions via PE: [128,2] = ones1^T @ s2
        pb = ps.tile([P, 2], F32, tag="pb")
        nc.tensor.matmul(pb[:], ones1[:], s2[:])
        invb = work.tile([P, 2], F32, tag="invb")
        nc.scalar.copy(invb[:], pb[:])

        # ================= phase 5: normalize + masked mean =================
        # a0 on ACT (scale is a per-partition AP), a1 on DVE — parallel.
        a0 = work.tile([P, H], FP16, tag="a0")
        nc.scalar.activation(a0[:], ys[0][:], AF.Copy, scale=invb[:, 0:1])
        a1 = work.tile([P, H], FP16, tag="a1")
        nc.vector.tensor_scalar(a1[:], ys[1][:], invb[:, 1:2], None, ALU.mult)
        mk = work.tile([P, H], FP16, tag="mk")
        nc.vector.tensor_tensor(mk[:], a0[:], a1[:], ALU.min)
        df = work.tile([P, H], FP16, tag="df")
        nc.vector.tensor_tensor(df[:], a0[:], a1[:], ALU.subtract)
        da = work.tile([P, H], FP16, tag="da")
        nc.scalar.activation(da[:], df[:], AF.Abs)
        nc.vector.tensor_scalar(mk[:], mk[:], 0.1, None, ALU.is_lt)
        s12 = work.tile([P, 2], F32, tag="s12")
        mdf = work.tile([P, H], FP16, tag="mdf")
        nc.vector.tensor_tensor(mdf[:], da[:], mk[:], ALU.mult)
        nc.vector.reduce_sum(s12[:, 0:1], mdf[:], axis=AX.X)
        nc.vector.reduce_sum(s12[:, 1:2], mk[:], axis=AX.X)
        # partition-dim sum via PE: [1,2] = ones[128,1]^T @ s12[128,2]
        pv = ps.tile([1, 2], F32, tag="pv")
        nc.tensor.matmul(pv[:], onesc[:], s12[:])
        pvs = work.tile([1, 2], F32, tag="pvs")
        nc.scalar.copy(pvs[:], pv[:])
        # keep the warm-up collective live: max-fold its zero output into
        # the (non-negative) partials, off the AllReduce critical path
        wb = work.tile([1, 2], F32, tag="wb")
        nc.sync.dma_start(wb[:], warm_out[0:1, 0:2])
        nc.vector.tensor_tensor(pvs[:], pvs[:], wb[:], ALU.max)
        nc.sync.dma_start(partials[:, :], pvs[:])


def _build(w_gt, w_pred):
    nc = bacc.Bacc("TRN2", target_bir_lowering=False, debug=False,
                   num_devices=NCORES)
    gts = nc.dram_tensor("gts", [P, NB * (P + 2 * w_gt)], FP16,
                         kind="ExternalInput")
    prs = nc.dram_tensor("prs", [P, NB * (P + 2 * w_pred)], FP16,
                         kind="ExternalInput")
    partials = nc.dram_tensor("partials", [1, 2], F32, kind="ExternalOutput")
    with tile.TileContext(nc) as tc:
        _body(tc, w_gt, w_pred, gts, prs, partials)
    nc.compile()
    return nc


_PROGRAMS = {}


def _program(*key):
    if key not in _PROGRAMS:
        _PROGRAMS[key] = _build(*key)
    return _PROGRAMS[key]


def _row_gmax(fg):
    """Max over pixels of the in-row distance to the nearest background
    pixel (clamped to BIG). This equals the exact column-pass window bound."""
    idx = np.arange(fg.shape[1], dtype=np.float64)
    zero = ~fg
    left = np.maximum.accumulate(np.where(zero, idx, -np.inf), axis=1)
    right = np.minimum.accumulate(np.where(zero, idx, np.inf)[:, ::-1],
                                  axis=1)[:, ::-1]
    g = np.minimum(np.minimum(idx - left, right - idx), BIG)
    return float(g.max())


def _bucket(gmax):
    need = min(int(np.ceil(gmax)), H - 1)
    for b in _BUCKETS:
        if b >= need:
            return b
    raise NotImplementedError(
        f"row gmax {gmax} exceeds the supported strip margin {_BUCKETS[-1]}")


def _strips(img, w):
    """Per-core fp16 strips [128, 8*(128+2w)]: strip[c][p, b*(128+2w)+q] =
    scaled img[128*b + p, 128*c - w + q], fg-padded outside the image."""
    x = np.asarray(img, np.float32) * 1e30
    pad = np.full((H, w), np.float32(1e30))
    xp = np.concatenate([pad, x, pad], axis=1)
    W = P + 2 * w
    out = []
    for c in range(NCORES):
        b = xp[:, c * P:c * P + W].astype(np.float16)
        out.append(np.ascontiguousarray(
            b.reshape(NB, P, W).transpose(1, 0, 2).reshape(P, NB * W)))
    return out


def _run(pred, gt, trace=False):
    pred = np.ascontiguousarray(np.asarray(pred), dtype=np.float32)
    gt = np.ascontiguousarray(np.asarray(gt), dtype=np.float32)
    assert pred.shape == (H, H) and gt.shape == (H, H)
    w_gt = _bucket(_row_gmax(gt != 0))
    w_pred = _bucket(_row_gmax(pred > 0))
    nc = _program(w_gt, w_pred)
    sg = _strips(gt, w_gt)
    sp = _strips(pred, w_pred)
    in_maps = [{"gts": sg[c], "prs": sp[c]} for c in range(NCORES)]
    res = run_bass_kernel_spmd(nc, in_maps, list(range(NCORES)), trace=trace)
    tot = np.zeros(2, np.float64)
    for r in res.results:
        tot += np.asarray(r["partials"], np.float64).reshape(-1)[:2]
    loss = np.float32(tot[0] / max(tot[1], 1.0))
    return loss, res


def kernel(pred, gt):
    loss, _ = _run(pred, gt)
    return loss
